# revision 8
# baseline (speedup 1.0000x reference)
"""Trainium2 Bass kernel for nn_MixtureAttention.

Math: the reference builds a (c,c) pairwise Cauchy-product matrix per batch,
row-normalizes it, and keeps only the diagonal.  `pi` cancels; with
    Q[i,p] = prod_d (sig[i,d]^2 + (mu[p,d]-mu[i,d])^2)
    S[i]   = s2prod[i] * sum_p 1/Q[i,p]        (s2prod = prod_d sig^2)
the kept diagonal is coef[i] = 1/S[i], and y[b,ch,c] = x[b,ch]*coef[b,c].

Key idea: Q[i,p] is a polynomial in mu[p,:]; with centered coords m'=mu-0.5
it separates into an 81-monomial feature contraction Q = G[i,:].F[p,:] on
the TensorEngine.  fp32 matmul measures ~2.05us/MM on HW, so instead each
operand is split 3-way in bf16 (8+8+8 mantissa bits) and the 6 cross-term
products with combined split-level <= 2 are kept — 486 K-rows, zero-padded
to 4 K=128 parts, accumulated into each PSUM chunk.  That's ~fp32 accuracy
at bf16 speed.  MMs are issued weight-major over half-blocks (one lhsT
part swept across 4 chunks before switching) — LDWEIGHTS reuse makes MMs
~2x faster than chunk-major order (116 vs 243 ns/MM measured).

The drain is a fused Reciprocal+row-sum pass (ACT Reciprocal via raw emit,
~1.2e-5 rel err; DVE takes a share via reciprocal_approx_fast+accum).

Numerics: rows are host-sorted by s2prod ascending; each core's first
128-row block (tiny s2prod = ill-conditioned) instead uses the 2+2 split
Q = (G01.F01)*(G23.F23) (9 features, 54 packed rows), and S is floored
at 1.  Validated vs fp64: metric ~1.4e-4 (gate 2e-2).

Sharding: core k handles batch k//2, c-half k%2 (2048 rows x 4096 points).
Host un-permutes output columns.
"""

import numpy as np

B, C, D, CH = 4, 4096, 4, 256
NCORES = 8
CW = C // 2            # rows per core
NBLK = CW // 128       # 16 row blocks
NP = 4                 # packed K-parts (486 rows -> 4 x 128)
HB = NBLK // 2         # blocks per epilogue half
HC = HB * 128          # columns per epilogue half

_cache = {}


def _build(bench_nrep=None, bench_span="full", parts=None):
    import concourse.bacc as bacc
    import concourse.mybir as mybir
    from concourse.tile import TileContext

    if parts is None:
        parts = {"block0", "main", "drain", "epi"}

    f32 = mybir.dt.float32
    bf = mybir.dt.bfloat16
    Alu = mybir.AluOpType
    Act = mybir.ActivationFunctionType

    nc = bacc.Bacc(None, target_bir_lowering=False)
    fp_r = nc.declare_dram_parameter("fp", [128, NP * C], bf, isOutput=False)
    gp_r = nc.declare_dram_parameter("gp", [128, NP * CW], bf, isOutput=False)
    fp0_r = nc.declare_dram_parameter("fp0", [54, 2 * C], bf, isOutput=False)
    gp0_r = nc.declare_dram_parameter("gp0", [54, 256], bf, isOutput=False)
    ps2_r = nc.declare_dram_parameter("ps2", [128, NBLK], f32, isOutput=False)
    xcol_r = nc.declare_dram_parameter("xcol", [128, 2], f32, isOutput=False)
    y = nc.declare_dram_parameter("y", [CH, CW], f32, isOutput=True)

    def recip_accum_act(in_ap, junk_ap, accum_ap):
        """ACT Reciprocal with free-dim row-sum accumulate (raw emit: the
        bass wrapper refuses Reciprocal; its table is ~1.2e-5 max rel err,
        fine for summing positive terms)."""
        eng = nc.scalar
        imm = lambda v: mybir.ImmediateValue(dtype=mybir.dt.float32, value=v)
        eng.add_instruction(
            mybir.InstActivation(
                name=nc.get_next_instruction_name(),
                func=Act.Reciprocal,
                ins=[eng.lower_ap(in_ap), imm(0.0), imm(1.0), imm(0.0)],
                outs=[eng.lower_ap(junk_ap), eng.lower_ap(accum_ap)],
            )
        )

    with TileContext(nc) as tc:
        with (
            tc.tile_pool(name="persist", bufs=1) as pp,
            tc.tile_pool(name="work", bufs=1) as wp,
            tc.tile_pool(name="psum", bufs=1, space="PSUM") as psp,
            tc.tile_pool(name="dram", bufs=1, space="DRAM") as dp,
        ):
            st = {}

            def loads():
                st["scr"] = dp.tile([CW], f32, name="scr", tag="scr", bufs=2)
                fp = st["fp"] = pp.tile([128, NP * C], bf, tag="fp", bufs=2, name="fp")
                gp = st["gp"] = pp.tile([128, NP * CW], bf, tag="gp", bufs=2, name="gp")
                fp0 = st["fp0"] = pp.tile([54, 2 * C], bf, tag="fp0", bufs=2, name="fp0")
                gp0 = st["gp0"] = pp.tile([54, 256], bf, tag="gp0", bufs=2, name="gp0")
                ps2_sb = st["ps2"] = pp.tile([128, NBLK], f32, tag="ps2s", bufs=2, name="ps2s")
                xcol = st["xcol"] = pp.tile([128, 2], f32, tag="xc", bufs=2, name="xc")
                Racc = st["Racc"] = pp.tile([128, NBLK, 4], f32, tag="Ra", bufs=2, name="Ra")
                st["Racc0"] = pp.tile([128, C // 512], f32, tag="Ra0", bufs=2, name="Ra0")
                for q in range(NP):
                    nc.sync.dma_start(
                        out=fp[:, q * C : (q + 1) * C],
                        in_=fp_r[:, q * C : (q + 1) * C],
                    )
                for q in range(2):
                    w = NP * CW // 2
                    nc.sync.dma_start(
                        out=gp[:, q * w : (q + 1) * w],
                        in_=gp_r[:, q * w : (q + 1) * w],
                    )
                nc.sync.dma_start(out=fp0[:, :], in_=fp0_r[:, :])
                nc.sync.dma_start(out=gp0[:, :], in_=gp0_r[:, :])
                nc.sync.dma_start(out=ps2_sb[:, :], in_=ps2_r[:, :])
                nc.sync.dma_start(out=xcol[:, :], in_=xcol_r[:, :])
                nc.vector.memset(Racc[:, 0, :], 0.0)

            def drain_main(qt, n, g):
                # qt: (128, 1024) PSUM group = points [g*1024, (g+1)*1024)
                Racc = st["Racc"]
                if (n * 2 + g) % 8 < 5:
                    junkC = wp.tile([128, 1024], f32, tag="jC", bufs=2, name="jC")
                    recip_accum_act(qt[:, :], junkC[:, :], Racc[:, n, g : g + 1])
                else:
                    r = wp.tile([128, 1024], f32, tag="r", bufs=2, name="r")
                    nc.vector.reciprocal_approx_fast(out=r[:, :], in_=qt[:, :])
                    junkD = wp.tile([128, 1024], f32, tag="jD", bufs=2, name="jD")
                    nc.vector.tensor_scalar(
                        junkD[:, :], r[:, :], 0.0, None, Alu.add, Alu.add,
                        accum_out=Racc[:, n, g : g + 1],
                    )

            def block0():
                fp0, gp0, Racc0 = st["fp0"], st["gp0"], st["Racc0"]
                # rows 0-127 (smallest s2prod): Q = (G01.F01)*(G23.F23)
                for j in range(C // 512):
                    sl = slice(j * 512, (j + 1) * 512)
                    sl2 = slice(C + j * 512, C + (j + 1) * 512)
                    qt = psp.tile([128, 1024], f32, tag="q", bufs=4, name="qp")
                    nc.tensor.matmul(
                        qt[:, 0:512], gp0[:, 0:128], fp0[:, sl],
                        start=True, stop=True,
                    )
                    nc.tensor.matmul(
                        qt[:, 512:1024], gp0[:, 128:256], fp0[:, sl2],
                        start=True, stop=True,
                    )
                    if "drain" in parts:
                        rA = wp.tile([128, 512], f32, tag="rA", bufs=2, name="rA")
                        junkA = wp.tile([128, 512], f32, tag="jA", bufs=2, name="jA")
                        recip_accum_act(qt[:, 0:512], rA[:, :], junkA[:, 0:1])
                        rB = wp.tile([128, 512], f32, tag="rB", bufs=2, name="rB")
                        nc.vector.reciprocal_approx_fast(
                            out=rB[:, :], in_=qt[:, 512:1024]
                        )
                        junkB = wp.tile([128, 512], f32, tag="jB", bufs=2, name="jB")
                        nc.vector.affine_mul_reduce(
                            out=junkB[:, :], accum_out=Racc0[:, j : j + 1],
                            in0=rA[:, :], in1=rB[:, :], scale=1.0, bias=0.0,
                        )

            def main_blocks(n_lo, n_hi):
                fp, gp = st["fp"], st["gp"]
                for n in range(n_lo, n_hi):
                    for hb in range(2):
                        # half-block: 4 chunks of 512 points = 2 PSUM tiles
                        pss = [
                            psp.tile([128, 1024], f32, tag="q", bufs=4, name="qt")
                            for _ in range(2)
                        ]
                        for q in range(NP):
                            gsl = slice(q * CW + n * 128, q * CW + (n + 1) * 128)
                            for j in range(4):
                                j0 = hb * 4 + j
                                nc.tensor.matmul(
                                    pss[j // 2][:, (j % 2) * 512 : (j % 2 + 1) * 512],
                                    gp[:, gsl],
                                    fp[:, q * C + j0 * 512 : q * C + (j0 + 1) * 512],
                                    start=(q == 0),
                                    stop=(q == NP - 1),
                                )
                        if "drain" in parts:
                            drain_main(pss[0], n, hb * 2)
                            drain_main(pss[1], n, hb * 2 + 1)

            def epilogue(half):
                scr, Racc, Racc0 = st["scr"], st["Racc"], st["Racc0"]
                ps2_sb, xcol = st["ps2"], st["xcol"]
                nsl = slice(half * HB, (half + 1) * HB)
                t1 = wp.tile([128, HB], f32, tag="t1", bufs=2, name="t1")
                nc.vector.tensor_tensor(
                    t1[:, :], Racc[:, nsl, 0], Racc[:, nsl, 1], Alu.add
                )
                t2 = wp.tile([128, HB], f32, tag="t2", bufs=2, name="t2")
                nc.vector.tensor_tensor(
                    t2[:, :], Racc[:, nsl, 2], Racc[:, nsl, 3], Alu.add
                )
                Rsum = wp.tile([128, HB], f32, tag="Rs", bufs=2, name="Rs")
                nc.vector.tensor_tensor(Rsum[:, :], t1[:, :], t2[:, :], Alu.add)
                if half == 0:
                    jr = wp.tile([128, C // 512], f32, tag="jr", bufs=2, name="jr")
                    nc.vector.tensor_scalar(
                        jr[:, :], Racc0[:, :], 0.0, None, Alu.add, Alu.add,
                        accum_out=Rsum[:, 0:1],
                    )
                S = wp.tile([128, HB], f32, tag="S", bufs=2, name="S")
                nc.vector.tensor_tensor(S[:, :], Rsum[:, :], ps2_sb[:, nsl], Alu.mult)
                nc.vector.tensor_scalar_max(S[:, :], S[:, :], 1.0)
                coef = wp.tile([128, HB], f32, tag="coef", bufs=2, name="coef")
                nc.vector.reciprocal(coef[:, :], S[:, :])

                # transpose (128, HB) -> c-ordered row via DRAM bounce
                nc.gpsimd.dma_start(
                    out=scr.rearrange("(n p) -> p n", p=128)[:, nsl], in_=coef[:, :]
                )
                cbc = wp.tile([128, HC], f32, tag="cbc", bufs=2, name="cbc")
                nc.gpsimd.dma_start(
                    out=cbc[:, :],
                    in_=scr.rearrange("(one c) -> one c", one=1)[
                        0:1, half * HC : (half + 1) * HC
                    ].broadcast_to([128, HC]),
                )
                for h in range(CH // 128):
                    zt = wp.tile([128, HC], f32, tag="zt", bufs=2, name="zt")
                    nc.vector.tensor_scalar_mul(zt[:, :], cbc[:, :], xcol[:, h : h + 1])
                    nc.gpsimd.dma_start(
                        out=y[h * 128 : (h + 1) * 128, half * HC : (half + 1) * HC],
                        in_=zt[:, :],
                    )

            def whole():
                loads()
                if "block0" in parts:
                    block0()
                if "main" in parts:
                    main_blocks(1, HB)
                if "epi" in parts and "drain" in parts:
                    epilogue(0)
                if "main" in parts:
                    main_blocks(HB, NBLK)
                if "epi" in parts and "drain" in parts:
                    epilogue(1)

            if bench_nrep is None:
                whole()
            else:
                import concourse.mybir as _mb

                with tc.For_i(
                    0, bench_nrep, 1,
                    staggered_reset=True,
                    hint_engines=(_mb.EngineType.DVE, _mb.EngineType.Activation),
                ):
                    whole()
    nc.finalize()
    return nc


def _get_nc():
    if "nc" not in _cache:
        _cache["nc"] = _build()
    return _cache["nc"]


_IDX4 = np.indices((3, 3, 3, 3)).reshape(4, -1).T  # (81, 4) exponent tuples
_IDX2 = np.indices((3, 3)).reshape(2, -1).T        # (9, 2)
_COMBOS = [(0, 0), (0, 1), (1, 0), (0, 2), (1, 1), (2, 0)]  # split levels i+j<=2


def _feat(m, s2, dims, idx):
    """G (rows, nf), F (points, nf) in float64 for the given dims."""
    n = m.shape[0]
    G = np.ones((n, len(idx)))
    F = np.ones((n, len(idx)))
    for e, exps in enumerate(idx):
        for d, ed in zip(dims, exps):
            gd = [s2[:, d] + m[:, d] ** 2, -2.0 * m[:, d], np.ones(n)][ed]
            fd = [np.ones(n), m[:, d], m[:, d] ** 2][ed]
            G[:, e] = G[:, e] * gd
            F[:, e] = F[:, e] * fd
    return G, F


def _bf16(a):
    bits = np.asarray(a, np.float32).view(np.uint32)
    r = ((bits.astype(np.uint64) + 0x7FFF + ((bits >> 16) & 1)) >> 16) << 16
    return r.astype(np.uint32).view(np.float32)


def _split3(a):
    a = np.asarray(a, np.float32)
    h = _bf16(a)
    m = _bf16((a - h).astype(np.float32))
    l = _bf16((a - h - m).astype(np.float32))
    return h, m, l


def _pack6(G, F):
    """bf16 3-way split, 6 cross-terms: (rows, 6nf), (points, 6nf)."""
    Gs = _split3(G.astype(np.float32))
    Fs = _split3(F.astype(np.float32))
    Gp = np.concatenate([Gs[i] for i, j in _COMBOS], axis=1)
    Fp = np.concatenate([Fs[j] for i, j in _COMBOS], axis=1)
    return Gp, Fp


def _to_parts(a, nrows, width):
    """(K, width) -> zero-pad K to NP*128 -> (128, NP*width) bf16."""
    import ml_dtypes

    pad = np.zeros((NP * 128, width), np.float32)
    pad[:nrows] = a
    return np.ascontiguousarray(
        pad.reshape(NP, 128, width).transpose(1, 0, 2).reshape(128, NP * width)
    ).astype(ml_dtypes.bfloat16)


def _in_maps(x, mu, sig):
    import ml_dtypes

    maps = []
    perms = []
    for k in range(NCORES):
        b, half = k // 2, k % 2
        m = mu[b].astype(np.float64) - 0.5        # centered, (C, D)
        s2 = sig[b].astype(np.float64) ** 2
        s2p = s2.prod(axis=1)
        order = np.argsort(
            s2p[half * CW : (half + 1) * CW], kind="stable"
        )  # within-half ascending s2prod
        rows = half * CW + order
        G, F = _feat(m, s2, (0, 1, 2, 3), _IDX4)
        G01, F01 = _feat(m, s2, (0, 1), _IDX2)
        G23, F23 = _feat(m, s2, (2, 3), _IDX2)
        r0 = rows[:128]
        Gp, Fp = _pack6(G[rows], F)               # (2048, 486), (4096, 486)
        Gp0, Fp0 = _pack6(G01[r0], F01)           # (128, 54), (4096, 54)
        Gp2, Fp2 = _pack6(G23[r0], F23)
        maps.append(
            {
                "fp": _to_parts(Fp.T, 486, C),
                "gp": _to_parts(Gp.T, 486, CW),
                "fp0": np.ascontiguousarray(
                    np.concatenate([Fp0.T, Fp2.T], axis=1)
                ).astype(ml_dtypes.bfloat16),
                "gp0": np.ascontiguousarray(
                    np.concatenate([Gp0.T, Gp2.T], axis=1)
                ).astype(ml_dtypes.bfloat16),
                "ps2": np.ascontiguousarray(
                    s2p[rows].reshape(NBLK, 128).T, np.float32
                ),
                "xcol": np.ascontiguousarray(
                    x[b, :, 0].reshape(2, 128).T, np.float32
                ),
            }
        )
        perms.append(order)
    return maps, perms


def kernel(x, pi, mu, sig):
    from concourse.bass_utils import run_bass_kernel_spmd

    nc = _get_nc()
    maps, perms = _in_maps(x, mu, sig)
    res = run_bass_kernel_spmd(nc, maps, list(range(NCORES))).results
    y = np.empty((B, CH, C), np.float32)
    for k in range(NCORES):
        b, half = k // 2, k % 2
        y[b][:, half * CW + perms[k]] = res[k]["y"]
    return y


# revision 9
# speedup vs baseline: 1.1160x; 1.1160x over previous
"""Trainium2 Bass kernel for nn_MixtureAttention.

Math: the reference builds a (c,c) pairwise Cauchy-product matrix per batch,
row-normalizes it, and keeps only the diagonal.  `pi` cancels; with
    Q[i,p] = prod_d (sig[i,d]^2 + (mu[p,d]-mu[i,d])^2)
    S[i]   = s2prod[i] * sum_p 1/Q[i,p]        (s2prod = prod_d sig^2)
the kept diagonal is coef[i] = 1/S[i], and y[b,ch,c] = x[b,ch]*coef[b,c].

Key idea: Q[i,p] is a polynomial in mu[p,:]; with centered coords m'=mu-0.5
it separates into an 81-monomial feature contraction Q = G[i,:].F[p,:] on
the TensorEngine.  fp32 matmul measures ~2.05us/MM on HW, so instead each
operand is split 3-way in bf16 (8+8+8 mantissa bits) and the 6 cross-term
products with combined split-level <= 2 are kept — 486 K-rows, zero-padded
to 4 K=128 parts, accumulated into each PSUM chunk.  That's ~fp32 accuracy
at bf16 speed.  MMs are issued weight-major over half-blocks (one lhsT
part swept across 4 chunks before switching) — LDWEIGHTS reuse makes MMs
~2x faster than chunk-major order (116 vs 243 ns/MM measured).

The drain is a fused Reciprocal+row-sum pass (ACT Reciprocal via raw emit,
~1.2e-5 rel err; DVE takes a share via reciprocal_approx_fast+accum).

Numerics: rows are host-sorted by s2prod ascending; each core's first
128-row block (tiny s2prod = ill-conditioned) instead uses the 2+2 split
Q = (G01.F01)*(G23.F23) (9 features, 54 packed rows), and S is floored
at 1.  Validated vs fp64: metric ~1.4e-4 (gate 2e-2).

Sharding: core k handles batch k//2, c-half k%2 (2048 rows x 4096 points).
Host un-permutes output columns.
"""

import numpy as np

B, C, D, CH = 4, 4096, 4, 256
NCORES = 8
CW = C // 2            # rows per core
NBLK = CW // 128       # 16 row blocks
NP = 4                 # packed K-parts (486 rows -> 4 x 128)
HB = NBLK // 2         # blocks per epilogue half
HC = HB * 128          # columns per epilogue half

_cache = {}


def _build(bench_nrep=None, bench_span="full", parts=None):
    import concourse.bacc as bacc
    import concourse.mybir as mybir
    from concourse.tile import TileContext

    if parts is None:
        parts = {"block0", "main", "drain", "epi"}

    f32 = mybir.dt.float32
    bf = mybir.dt.bfloat16
    Alu = mybir.AluOpType
    Act = mybir.ActivationFunctionType

    nc = bacc.Bacc(None, target_bir_lowering=False)
    fp_r = nc.declare_dram_parameter("fp", [128, NP * C], bf, isOutput=False)
    gp_r = nc.declare_dram_parameter("gp", [128, NP * CW], bf, isOutput=False)
    fp0_r = nc.declare_dram_parameter("fp0", [54, 2 * C], bf, isOutput=False)
    gp0_r = nc.declare_dram_parameter("gp0", [54, 256], bf, isOutput=False)
    ps2_r = nc.declare_dram_parameter("ps2", [128, NBLK], f32, isOutput=False)
    xcol_r = nc.declare_dram_parameter("xcol", [128, 2], f32, isOutput=False)
    y = nc.declare_dram_parameter("y", [CH, CW], f32, isOutput=True)

    def recip_accum_act(in_ap, junk_ap, accum_ap):
        """ACT Reciprocal with free-dim row-sum accumulate (raw emit: the
        bass wrapper refuses Reciprocal; its table is ~1.2e-5 max rel err,
        fine for summing positive terms)."""
        eng = nc.scalar
        imm = lambda v: mybir.ImmediateValue(dtype=mybir.dt.float32, value=v)
        eng.add_instruction(
            mybir.InstActivation(
                name=nc.get_next_instruction_name(),
                func=Act.Reciprocal,
                ins=[eng.lower_ap(in_ap), imm(0.0), imm(1.0), imm(0.0)],
                outs=[eng.lower_ap(junk_ap), eng.lower_ap(accum_ap)],
            )
        )

    with TileContext(nc) as tc:
        with (
            tc.tile_pool(name="persist", bufs=1) as pp,
            tc.tile_pool(name="work", bufs=1) as wp,
            tc.tile_pool(name="psum", bufs=1, space="PSUM") as psp,
            tc.tile_pool(name="dram", bufs=1, space="DRAM") as dp,
        ):
            st = {}

            def loads():
                st["scr"] = dp.tile([CW], f32, name="scr", tag="scr", bufs=2)
                fp = st["fp"] = pp.tile([128, NP * C], bf, tag="fp", bufs=2, name="fp")
                gp = st["gp"] = pp.tile([128, NP * CW], bf, tag="gp", bufs=2, name="gp")
                fp0 = st["fp0"] = pp.tile([54, 2 * C], bf, tag="fp0", bufs=2, name="fp0")
                gp0 = st["gp0"] = pp.tile([54, 256], bf, tag="gp0", bufs=2, name="gp0")
                ps2_sb = st["ps2"] = pp.tile([128, NBLK], f32, tag="ps2s", bufs=2, name="ps2s")
                xcol = st["xcol"] = pp.tile([128, 2], f32, tag="xc", bufs=2, name="xc")
                Racc = st["Racc"] = pp.tile([128, NBLK, 4], f32, tag="Ra", bufs=2, name="Ra")
                st["Racc0"] = pp.tile([128, C // 512], f32, tag="Ra0", bufs=2, name="Ra0")
                for q in range(NP):
                    nc.sync.dma_start(
                        out=fp[:, q * C : (q + 1) * C],
                        in_=fp_r[:, q * C : (q + 1) * C],
                    )
                for q in range(2):
                    w = NP * CW // 2
                    nc.sync.dma_start(
                        out=gp[:, q * w : (q + 1) * w],
                        in_=gp_r[:, q * w : (q + 1) * w],
                    )
                nc.sync.dma_start(out=fp0[:, :], in_=fp0_r[:, :])
                nc.sync.dma_start(out=gp0[:, :], in_=gp0_r[:, :])
                nc.sync.dma_start(out=ps2_sb[:, :], in_=ps2_r[:, :])
                nc.sync.dma_start(out=xcol[:, :], in_=xcol_r[:, :])
                nc.vector.memset(Racc[:, 0, :], 0.0)

            def drain_main(qt, n, g):
                # qt: (128, 1024) PSUM group = points [g*1024, (g+1)*1024)
                Racc = st["Racc"]
                if (n * 2 + g) % 8 < 5:
                    junkC = wp.tile([128, 1024], f32, tag="jC", bufs=2, name="jC")
                    recip_accum_act(qt[:, :], junkC[:, :], Racc[:, n, g : g + 1])
                else:
                    r = wp.tile([128, 1024], f32, tag="r", bufs=2, name="r")
                    nc.vector.reciprocal_approx_fast(out=r[:, :], in_=qt[:, :])
                    junkD = wp.tile([128, 1024], f32, tag="jD", bufs=2, name="jD")
                    nc.vector.tensor_scalar(
                        junkD[:, :], r[:, :], 0.0, None, Alu.add, Alu.add,
                        accum_out=Racc[:, n, g : g + 1],
                    )

            def block0():
                fp0, gp0, Racc0 = st["fp0"], st["gp0"], st["Racc0"]
                # rows 0-127 (smallest s2prod): Q = (G01.F01)*(G23.F23)
                for j in range(C // 512):
                    sl = slice(j * 512, (j + 1) * 512)
                    sl2 = slice(C + j * 512, C + (j + 1) * 512)
                    qt = psp.tile([128, 1024], f32, tag="q", bufs=4, name="qp")
                    nc.tensor.matmul(
                        qt[:, 0:512], gp0[:, 0:128], fp0[:, sl],
                        start=True, stop=True,
                    )
                    nc.tensor.matmul(
                        qt[:, 512:1024], gp0[:, 128:256], fp0[:, sl2],
                        start=True, stop=True,
                    )
                    if "drain" in parts:
                        rA = wp.tile([128, 512], f32, tag="rA", bufs=2, name="rA")
                        junkA = wp.tile([128, 512], f32, tag="jA", bufs=2, name="jA")
                        recip_accum_act(qt[:, 0:512], rA[:, :], junkA[:, 0:1])
                        rB = wp.tile([128, 512], f32, tag="rB", bufs=2, name="rB")
                        nc.vector.reciprocal_approx_fast(
                            out=rB[:, :], in_=qt[:, 512:1024]
                        )
                        junkB = wp.tile([128, 512], f32, tag="jB", bufs=2, name="jB")
                        nc.vector.affine_mul_reduce(
                            out=junkB[:, :], accum_out=Racc0[:, j : j + 1],
                            in0=rA[:, :], in1=rB[:, :], scale=1.0, bias=0.0,
                        )

            def main_blocks(n_lo, n_hi):
                fp, gp = st["fp"], st["gp"]
                for n in range(n_lo, n_hi):
                    for hb in range(2):
                        # half-block: 4 chunks of 512 points = 2 PSUM tiles
                        pss = [
                            psp.tile([128, 1024], f32, tag="q", bufs=4, name="qt")
                            for _ in range(2)
                        ]
                        for q in range(NP):
                            gsl = slice(q * CW + n * 128, q * CW + (n + 1) * 128)
                            for j in range(4):
                                j0 = hb * 4 + j
                                nc.tensor.matmul(
                                    pss[j // 2][:, (j % 2) * 512 : (j % 2 + 1) * 512],
                                    gp[:, gsl],
                                    fp[:, q * C + j0 * 512 : q * C + (j0 + 1) * 512],
                                    start=(q == 0),
                                    stop=(q == NP - 1),
                                )
                        if "drain" in parts:
                            drain_main(pss[0], n, hb * 2)
                            drain_main(pss[1], n, hb * 2 + 1)

            def epilogue(half):
                scr, Racc, Racc0 = st["scr"], st["Racc"], st["Racc0"]
                ps2_sb, xcol = st["ps2"], st["xcol"]
                nsl = slice(half * HB, (half + 1) * HB)
                t1 = wp.tile([128, HB], f32, tag="t1", bufs=2, name="t1")
                nc.vector.tensor_tensor(
                    t1[:, :], Racc[:, nsl, 0], Racc[:, nsl, 1], Alu.add
                )
                t2 = wp.tile([128, HB], f32, tag="t2", bufs=2, name="t2")
                nc.vector.tensor_tensor(
                    t2[:, :], Racc[:, nsl, 2], Racc[:, nsl, 3], Alu.add
                )
                Rsum = wp.tile([128, HB], f32, tag="Rs", bufs=2, name="Rs")
                nc.vector.tensor_tensor(Rsum[:, :], t1[:, :], t2[:, :], Alu.add)
                if half == 0:
                    jr = wp.tile([128, C // 512], f32, tag="jr", bufs=2, name="jr")
                    nc.vector.tensor_scalar(
                        jr[:, :], Racc0[:, :], 0.0, None, Alu.add, Alu.add,
                        accum_out=Rsum[:, 0:1],
                    )
                S = wp.tile([128, HB], f32, tag="S", bufs=2, name="S")
                nc.vector.tensor_tensor(S[:, :], Rsum[:, :], ps2_sb[:, nsl], Alu.mult)
                nc.vector.tensor_scalar_max(S[:, :], S[:, :], 1.0)
                coef = wp.tile([128, HB], f32, tag="coef", bufs=2, name="coef")
                nc.vector.reciprocal(coef[:, :], S[:, :])

                # transpose (128, HB) -> c-ordered row via DRAM bounce
                nc.sync.dma_start(
                    out=scr.rearrange("(n p) -> p n", p=128)[:, nsl], in_=coef[:, :]
                )
                cbc = wp.tile([128, HC], f32, tag="cbc", bufs=2, name="cbc")
                nc.sync.dma_start(
                    out=cbc[:, :],
                    in_=scr.rearrange("(one c) -> one c", one=1)[
                        0:1, half * HC : (half + 1) * HC
                    ].broadcast_to([128, HC]),
                )
                for h in range(CH // 128):
                    zt = wp.tile([128, HC], f32, tag="zt", bufs=2, name="zt")
                    nc.vector.tensor_scalar_mul(zt[:, :], cbc[:, :], xcol[:, h : h + 1])
                    nc.sync.dma_start(
                        out=y[h * 128 : (h + 1) * 128, half * HC : (half + 1) * HC],
                        in_=zt[:, :],
                    )

            def whole():
                loads()
                if "block0" in parts:
                    block0()
                if "main" in parts:
                    main_blocks(1, HB)
                if "epi" in parts and "drain" in parts:
                    epilogue(0)
                if "main" in parts:
                    main_blocks(HB, NBLK)
                if "epi" in parts and "drain" in parts:
                    epilogue(1)

            if bench_nrep is None:
                whole()
            else:
                import concourse.mybir as _mb

                with tc.For_i(
                    0, bench_nrep, 1,
                    staggered_reset=True,
                    hint_engines=(_mb.EngineType.DVE, _mb.EngineType.Activation),
                ):
                    whole()
    nc.finalize()
    return nc


def _get_nc():
    if "nc" not in _cache:
        _cache["nc"] = _build()
    return _cache["nc"]


_IDX4 = np.indices((3, 3, 3, 3)).reshape(4, -1).T  # (81, 4) exponent tuples
_IDX2 = np.indices((3, 3)).reshape(2, -1).T        # (9, 2)
_COMBOS = [(0, 0), (0, 1), (1, 0), (0, 2), (1, 1), (2, 0)]  # split levels i+j<=2


def _feat(m, s2, dims, idx):
    """G (rows, nf), F (points, nf) in float64 for the given dims."""
    n = m.shape[0]
    G = np.ones((n, len(idx)))
    F = np.ones((n, len(idx)))
    for e, exps in enumerate(idx):
        for d, ed in zip(dims, exps):
            gd = [s2[:, d] + m[:, d] ** 2, -2.0 * m[:, d], np.ones(n)][ed]
            fd = [np.ones(n), m[:, d], m[:, d] ** 2][ed]
            G[:, e] = G[:, e] * gd
            F[:, e] = F[:, e] * fd
    return G, F


def _bf16(a):
    bits = np.asarray(a, np.float32).view(np.uint32)
    r = ((bits.astype(np.uint64) + 0x7FFF + ((bits >> 16) & 1)) >> 16) << 16
    return r.astype(np.uint32).view(np.float32)


def _split3(a):
    a = np.asarray(a, np.float32)
    h = _bf16(a)
    m = _bf16((a - h).astype(np.float32))
    l = _bf16((a - h - m).astype(np.float32))
    return h, m, l


def _pack6(G, F):
    """bf16 3-way split, 6 cross-terms: (rows, 6nf), (points, 6nf)."""
    Gs = _split3(G.astype(np.float32))
    Fs = _split3(F.astype(np.float32))
    Gp = np.concatenate([Gs[i] for i, j in _COMBOS], axis=1)
    Fp = np.concatenate([Fs[j] for i, j in _COMBOS], axis=1)
    return Gp, Fp


def _to_parts(a, nrows, width):
    """(K, width) -> zero-pad K to NP*128 -> (128, NP*width) bf16."""
    import ml_dtypes

    pad = np.zeros((NP * 128, width), np.float32)
    pad[:nrows] = a
    return np.ascontiguousarray(
        pad.reshape(NP, 128, width).transpose(1, 0, 2).reshape(128, NP * width)
    ).astype(ml_dtypes.bfloat16)


def _in_maps(x, mu, sig):
    import ml_dtypes

    maps = []
    perms = []
    for k in range(NCORES):
        b, half = k // 2, k % 2
        m = mu[b].astype(np.float64) - 0.5        # centered, (C, D)
        s2 = sig[b].astype(np.float64) ** 2
        s2p = s2.prod(axis=1)
        order = np.argsort(
            s2p[half * CW : (half + 1) * CW], kind="stable"
        )  # within-half ascending s2prod
        rows = half * CW + order
        G, F = _feat(m, s2, (0, 1, 2, 3), _IDX4)
        G01, F01 = _feat(m, s2, (0, 1), _IDX2)
        G23, F23 = _feat(m, s2, (2, 3), _IDX2)
        r0 = rows[:128]
        Gp, Fp = _pack6(G[rows], F)               # (2048, 486), (4096, 486)
        Gp0, Fp0 = _pack6(G01[r0], F01)           # (128, 54), (4096, 54)
        Gp2, Fp2 = _pack6(G23[r0], F23)
        maps.append(
            {
                "fp": _to_parts(Fp.T, 486, C),
                "gp": _to_parts(Gp.T, 486, CW),
                "fp0": np.ascontiguousarray(
                    np.concatenate([Fp0.T, Fp2.T], axis=1)
                ).astype(ml_dtypes.bfloat16),
                "gp0": np.ascontiguousarray(
                    np.concatenate([Gp0.T, Gp2.T], axis=1)
                ).astype(ml_dtypes.bfloat16),
                "ps2": np.ascontiguousarray(
                    s2p[rows].reshape(NBLK, 128).T, np.float32
                ),
                "xcol": np.ascontiguousarray(
                    x[b, :, 0].reshape(2, 128).T, np.float32
                ),
            }
        )
        perms.append(order)
    return maps, perms


def kernel(x, pi, mu, sig):
    from concourse.bass_utils import run_bass_kernel_spmd

    nc = _get_nc()
    maps, perms = _in_maps(x, mu, sig)
    res = run_bass_kernel_spmd(nc, maps, list(range(NCORES))).results
    y = np.empty((B, CH, C), np.float32)
    for k in range(NCORES):
        b, half = k // 2, k % 2
        y[b][:, half * CW + perms[k]] = res[k]["y"]
    return y


# revision 10
# speedup vs baseline: 1.1896x; 1.0660x over previous
"""Trainium2 Bass kernel for nn_MixtureAttention.

Math: the reference builds a (c,c) pairwise Cauchy-product matrix per batch,
row-normalizes it, and keeps only the diagonal.  `pi` cancels; with
    Q[i,p] = prod_d (sig[i,d]^2 + (mu[p,d]-mu[i,d])^2)
    S[i]   = s2prod[i] * sum_p 1/Q[i,p]        (s2prod = prod_d sig^2)
the kept diagonal is coef[i] = 1/S[i], and y[b,ch,c] = x[b,ch]*coef[b,c].

Key idea: Q[i,p] is a polynomial in mu[p,:]; with centered coords m'=mu-0.5
it separates into an 81-monomial feature contraction Q = G[i,:].F[p,:] on
the TensorEngine.  fp32 matmul measures ~2.05us/MM on HW, so instead each
operand is split 3-way in bf16 (8+8+8 mantissa bits) and the 6 cross-term
products with combined split-level <= 2 are kept — 486 K-rows, zero-padded
to 4 K=128 parts, accumulated into each PSUM chunk.  That's ~fp32 accuracy
at bf16 speed.  MMs are issued weight-major over half-blocks (one lhsT
part swept across 4 chunks before switching) — LDWEIGHTS reuse makes MMs
~2x faster than chunk-major order (116 vs 243 ns/MM measured).

The drain is a fused Reciprocal+row-sum pass (ACT Reciprocal via raw emit,
~1.2e-5 rel err; DVE takes a share via reciprocal_approx_fast+accum).

Numerics: rows are host-sorted by s2prod ascending; each core's first
128-row block (tiny s2prod = ill-conditioned) instead uses the 2+2 split
Q = (G01.F01)*(G23.F23) (9 features, 54 packed rows), and S is floored
at 1.  Validated vs fp64: metric ~1.4e-4 (gate 2e-2).

Sharding: core k handles batch k//2, c-half k%2 (2048 rows x 4096 points).
Host un-permutes output columns.
"""

import numpy as np

B, C, D, CH = 4, 4096, 4, 256
NCORES = 8
CW = C // 2            # rows per core
NBLK = CW // 128       # 16 row blocks
NP = 4                 # packed K-parts (486 rows -> 4 x 128)
HB = NBLK // 2         # blocks per epilogue half
HC = HB * 128          # columns per epilogue half

_cache = {}


def _build(bench_nrep=None, bench_span="full", parts=None):
    import concourse.bacc as bacc
    import concourse.mybir as mybir
    from concourse.tile import TileContext

    if parts is None:
        parts = {"block0", "main", "drain", "epi"}

    f32 = mybir.dt.float32
    bf = mybir.dt.bfloat16
    Alu = mybir.AluOpType
    Act = mybir.ActivationFunctionType

    nc = bacc.Bacc(None, target_bir_lowering=False)
    fp_r = nc.declare_dram_parameter("fp", [128, NP * C], bf, isOutput=False)
    gp_r = nc.declare_dram_parameter("gp", [128, NP * CW], bf, isOutput=False)
    fp0_r = nc.declare_dram_parameter("fp0", [54, 2 * C], bf, isOutput=False)
    gp0_r = nc.declare_dram_parameter("gp0", [54, 256], bf, isOutput=False)
    ps2_r = nc.declare_dram_parameter("ps2", [128, NBLK], f32, isOutput=False)
    xcol_r = nc.declare_dram_parameter("xcol", [128, 2], f32, isOutput=False)
    y = nc.declare_dram_parameter("y", [CH, CW], f32, isOutput=True)

    def recip_accum_act(in_ap, junk_ap, accum_ap):
        """ACT Reciprocal with free-dim row-sum accumulate (raw emit: the
        bass wrapper refuses Reciprocal; its table is ~1.2e-5 max rel err,
        fine for summing positive terms)."""
        eng = nc.scalar
        imm = lambda v: mybir.ImmediateValue(dtype=mybir.dt.float32, value=v)
        eng.add_instruction(
            mybir.InstActivation(
                name=nc.get_next_instruction_name(),
                func=Act.Reciprocal,
                ins=[eng.lower_ap(in_ap), imm(0.0), imm(1.0), imm(0.0)],
                outs=[eng.lower_ap(junk_ap), eng.lower_ap(accum_ap)],
            )
        )

    with TileContext(nc) as tc:
        with (
            tc.tile_pool(name="persist", bufs=1) as pp,
            tc.tile_pool(name="work", bufs=1) as wp,
            tc.tile_pool(name="psum", bufs=1, space="PSUM") as psp,
            tc.tile_pool(name="dram", bufs=1, space="DRAM") as dp,
        ):
            st = {}

            def loads():
                st["scr"] = dp.tile([CW], f32, name="scr", tag="scr", bufs=2)
                fp = st["fp"] = pp.tile([128, NP * C], bf, tag="fp", bufs=2, name="fp")
                gp = st["gp"] = pp.tile([128, NP * CW], bf, tag="gp", bufs=2, name="gp")
                fp0 = st["fp0"] = pp.tile([54, 2 * C], bf, tag="fp0", bufs=2, name="fp0")
                gp0 = st["gp0"] = pp.tile([54, 256], bf, tag="gp0", bufs=2, name="gp0")
                ps2_sb = st["ps2"] = pp.tile([128, NBLK], f32, tag="ps2s", bufs=2, name="ps2s")
                xcol = st["xcol"] = pp.tile([128, 2], f32, tag="xc", bufs=2, name="xc")
                Racc = st["Racc"] = pp.tile([128, NBLK, 4], f32, tag="Ra", bufs=2, name="Ra")
                st["Racc0"] = pp.tile([128, C // 512], f32, tag="Ra0", bufs=2, name="Ra0")
                for q in range(NP):
                    nc.sync.dma_start(
                        out=fp[:, q * C : (q + 1) * C],
                        in_=fp_r[:, q * C : (q + 1) * C],
                    )
                for q in range(2):
                    w = NP * CW // 2
                    nc.sync.dma_start(
                        out=gp[:, q * w : (q + 1) * w],
                        in_=gp_r[:, q * w : (q + 1) * w],
                    )
                nc.sync.dma_start(out=fp0[:, :], in_=fp0_r[:, :])
                nc.sync.dma_start(out=gp0[:, :], in_=gp0_r[:, :])
                nc.sync.dma_start(out=ps2_sb[:, :], in_=ps2_r[:, :])
                nc.sync.dma_start(out=xcol[:, :], in_=xcol_r[:, :])
                nc.vector.memset(Racc[:, 0, :], 0.0)

            def drain_main(qt, n, g):
                # qt: (128, 1024) PSUM group = points [g*1024, (g+1)*1024)
                # ACT-only: measured 320 ns/group; DVE recip+accum is ~2.2us
                Racc = st["Racc"]
                junkC = wp.tile([128, 1024], f32, tag="jC", bufs=2, name="jC")
                recip_accum_act(qt[:, :], junkC[:, :], Racc[:, n, g : g + 1])

            def block0():
                fp0, gp0, Racc0 = st["fp0"], st["gp0"], st["Racc0"]
                # rows 0-127 (smallest s2prod): Q = (G01.F01)*(G23.F23)
                for j in range(C // 512):
                    sl = slice(j * 512, (j + 1) * 512)
                    sl2 = slice(C + j * 512, C + (j + 1) * 512)
                    qt = psp.tile([128, 1024], f32, tag="q", bufs=4, name="qp")
                    nc.tensor.matmul(
                        qt[:, 0:512], gp0[:, 0:128], fp0[:, sl],
                        start=True, stop=True,
                    )
                    nc.tensor.matmul(
                        qt[:, 512:1024], gp0[:, 128:256], fp0[:, sl2],
                        start=True, stop=True,
                    )
                    if "drain" in parts:
                        rA = wp.tile([128, 512], f32, tag="rA", bufs=2, name="rA")
                        junkA = wp.tile([128, 512], f32, tag="jA", bufs=2, name="jA")
                        recip_accum_act(qt[:, 0:512], rA[:, :], junkA[:, 0:1])
                        rB = wp.tile([128, 512], f32, tag="rB", bufs=2, name="rB")
                        nc.vector.reciprocal_approx_fast(
                            out=rB[:, :], in_=qt[:, 512:1024]
                        )
                        junkB = wp.tile([128, 512], f32, tag="jB", bufs=2, name="jB")
                        nc.vector.affine_mul_reduce(
                            out=junkB[:, :], accum_out=Racc0[:, j : j + 1],
                            in0=rA[:, :], in1=rB[:, :], scale=1.0, bias=0.0,
                        )

            def main_blocks(n_lo, n_hi):
                fp, gp = st["fp"], st["gp"]
                for n in range(n_lo, n_hi):
                    for hb in range(2):
                        # half-block: 4 chunks of 512 points = 2 PSUM tiles
                        pss = [
                            psp.tile([128, 1024], f32, tag="q", bufs=4, name="qt")
                            for _ in range(2)
                        ]
                        for q in range(NP):
                            gsl = slice(q * CW + n * 128, q * CW + (n + 1) * 128)
                            for j in range(4):
                                j0 = hb * 4 + j
                                nc.tensor.matmul(
                                    pss[j // 2][:, (j % 2) * 512 : (j % 2 + 1) * 512],
                                    gp[:, gsl],
                                    fp[:, q * C + j0 * 512 : q * C + (j0 + 1) * 512],
                                    start=(q == 0),
                                    stop=(q == NP - 1),
                                )
                        if "drain" in parts:
                            drain_main(pss[0], n, hb * 2)
                            drain_main(pss[1], n, hb * 2 + 1)

            def epilogue(half):
                scr, Racc, Racc0 = st["scr"], st["Racc"], st["Racc0"]
                ps2_sb, xcol = st["ps2"], st["xcol"]
                nsl = slice(half * HB, (half + 1) * HB)
                t1 = wp.tile([128, HB], f32, tag="t1", bufs=2, name="t1")
                nc.vector.tensor_tensor(
                    t1[:, :], Racc[:, nsl, 0], Racc[:, nsl, 1], Alu.add
                )
                t2 = wp.tile([128, HB], f32, tag="t2", bufs=2, name="t2")
                nc.vector.tensor_tensor(
                    t2[:, :], Racc[:, nsl, 2], Racc[:, nsl, 3], Alu.add
                )
                Rsum = wp.tile([128, HB], f32, tag="Rs", bufs=2, name="Rs")
                nc.vector.tensor_tensor(Rsum[:, :], t1[:, :], t2[:, :], Alu.add)
                if half == 0:
                    jr = wp.tile([128, C // 512], f32, tag="jr", bufs=2, name="jr")
                    nc.vector.tensor_scalar(
                        jr[:, :], Racc0[:, :], 0.0, None, Alu.add, Alu.add,
                        accum_out=Rsum[:, 0:1],
                    )
                S = wp.tile([128, HB], f32, tag="S", bufs=2, name="S")
                nc.vector.tensor_tensor(S[:, :], Rsum[:, :], ps2_sb[:, nsl], Alu.mult)
                nc.vector.tensor_scalar_max(S[:, :], S[:, :], 1.0)
                coef = wp.tile([128, HB], f32, tag="coef", bufs=2, name="coef")
                nc.vector.reciprocal(coef[:, :], S[:, :])

                # transpose (128, HB) -> c-ordered row via DRAM bounce
                nc.sync.dma_start(
                    out=scr.rearrange("(n p) -> p n", p=128)[:, nsl], in_=coef[:, :]
                )
                cbc = wp.tile([128, HC], f32, tag="cbc", bufs=2, name="cbc")
                nc.sync.dma_start(
                    out=cbc[:, :],
                    in_=scr.rearrange("(one c) -> one c", one=1)[
                        0:1, half * HC : (half + 1) * HC
                    ].broadcast_to([128, HC]),
                )
                for h in range(CH // 128):
                    zt = wp.tile([128, HC], f32, tag="zt", bufs=2, name="zt")
                    nc.vector.tensor_scalar_mul(zt[:, :], cbc[:, :], xcol[:, h : h + 1])
                    nc.sync.dma_start(
                        out=y[h * 128 : (h + 1) * 128, half * HC : (half + 1) * HC],
                        in_=zt[:, :],
                    )

            def whole():
                loads()
                if "block0" in parts:
                    block0()
                if "main" in parts:
                    main_blocks(1, HB)
                if "epi" in parts and "drain" in parts:
                    epilogue(0)
                if "main" in parts:
                    main_blocks(HB, NBLK)
                if "epi" in parts and "drain" in parts:
                    epilogue(1)

            if bench_nrep is None:
                whole()
            else:
                import concourse.mybir as _mb

                with tc.For_i(
                    0, bench_nrep, 1,
                    staggered_reset=True,
                    hint_engines=(_mb.EngineType.DVE, _mb.EngineType.Activation),
                ):
                    whole()
    nc.finalize()
    return nc


def _get_nc():
    if "nc" not in _cache:
        _cache["nc"] = _build()
    return _cache["nc"]


_IDX4 = np.indices((3, 3, 3, 3)).reshape(4, -1).T  # (81, 4) exponent tuples
_IDX2 = np.indices((3, 3)).reshape(2, -1).T        # (9, 2)
_COMBOS = [(0, 0), (0, 1), (1, 0), (0, 2), (1, 1), (2, 0)]  # split levels i+j<=2


def _feat(m, s2, dims, idx):
    """G (rows, nf), F (points, nf) in float64 for the given dims."""
    n = m.shape[0]
    G = np.ones((n, len(idx)))
    F = np.ones((n, len(idx)))
    for e, exps in enumerate(idx):
        for d, ed in zip(dims, exps):
            gd = [s2[:, d] + m[:, d] ** 2, -2.0 * m[:, d], np.ones(n)][ed]
            fd = [np.ones(n), m[:, d], m[:, d] ** 2][ed]
            G[:, e] = G[:, e] * gd
            F[:, e] = F[:, e] * fd
    return G, F


def _bf16(a):
    bits = np.asarray(a, np.float32).view(np.uint32)
    r = ((bits.astype(np.uint64) + 0x7FFF + ((bits >> 16) & 1)) >> 16) << 16
    return r.astype(np.uint32).view(np.float32)


def _split3(a):
    a = np.asarray(a, np.float32)
    h = _bf16(a)
    m = _bf16((a - h).astype(np.float32))
    l = _bf16((a - h - m).astype(np.float32))
    return h, m, l


def _pack6(G, F):
    """bf16 3-way split, 6 cross-terms: (rows, 6nf), (points, 6nf)."""
    Gs = _split3(G.astype(np.float32))
    Fs = _split3(F.astype(np.float32))
    Gp = np.concatenate([Gs[i] for i, j in _COMBOS], axis=1)
    Fp = np.concatenate([Fs[j] for i, j in _COMBOS], axis=1)
    return Gp, Fp


def _to_parts(a, nrows, width):
    """(K, width) -> zero-pad K to NP*128 -> (128, NP*width) bf16."""
    import ml_dtypes

    pad = np.zeros((NP * 128, width), np.float32)
    pad[:nrows] = a
    return np.ascontiguousarray(
        pad.reshape(NP, 128, width).transpose(1, 0, 2).reshape(128, NP * width)
    ).astype(ml_dtypes.bfloat16)


def _in_maps(x, mu, sig):
    import ml_dtypes

    maps = []
    perms = []
    for k in range(NCORES):
        b, half = k // 2, k % 2
        m = mu[b].astype(np.float64) - 0.5        # centered, (C, D)
        s2 = sig[b].astype(np.float64) ** 2
        s2p = s2.prod(axis=1)
        order = np.argsort(
            s2p[half * CW : (half + 1) * CW], kind="stable"
        )  # within-half ascending s2prod
        rows = half * CW + order
        G, F = _feat(m, s2, (0, 1, 2, 3), _IDX4)
        G01, F01 = _feat(m, s2, (0, 1), _IDX2)
        G23, F23 = _feat(m, s2, (2, 3), _IDX2)
        r0 = rows[:128]
        Gp, Fp = _pack6(G[rows], F)               # (2048, 486), (4096, 486)
        Gp0, Fp0 = _pack6(G01[r0], F01)           # (128, 54), (4096, 54)
        Gp2, Fp2 = _pack6(G23[r0], F23)
        maps.append(
            {
                "fp": _to_parts(Fp.T, 486, C),
                "gp": _to_parts(Gp.T, 486, CW),
                "fp0": np.ascontiguousarray(
                    np.concatenate([Fp0.T, Fp2.T], axis=1)
                ).astype(ml_dtypes.bfloat16),
                "gp0": np.ascontiguousarray(
                    np.concatenate([Gp0.T, Gp2.T], axis=1)
                ).astype(ml_dtypes.bfloat16),
                "ps2": np.ascontiguousarray(
                    s2p[rows].reshape(NBLK, 128).T, np.float32
                ),
                "xcol": np.ascontiguousarray(
                    x[b, :, 0].reshape(2, 128).T, np.float32
                ),
            }
        )
        perms.append(order)
    return maps, perms


def kernel(x, pi, mu, sig):
    from concourse.bass_utils import run_bass_kernel_spmd

    nc = _get_nc()
    maps, perms = _in_maps(x, mu, sig)
    res = run_bass_kernel_spmd(nc, maps, list(range(NCORES))).results
    y = np.empty((B, CH, C), np.float32)
    for k in range(NCORES):
        b, half = k // 2, k % 2
        y[b][:, half * CW + perms[k]] = res[k]["y"]
    return y


# revision 12
# speedup vs baseline: 1.2178x; 1.0237x over previous
"""Trainium2 Bass kernel for nn_MixtureAttention.

Math: the reference builds a (c,c) pairwise Cauchy-product matrix per batch,
row-normalizes it, and keeps only the diagonal.  `pi` cancels; with
    Q[i,p] = prod_d (sig[i,d]^2 + (mu[p,d]-mu[i,d])^2)
    S[i]   = s2prod[i] * sum_p 1/Q[i,p]        (s2prod = prod_d sig^2)
the kept diagonal is coef[i] = 1/S[i], and y[b,ch,c] = x[b,ch]*coef[b,c].

Key idea: Q[i,p] is a polynomial in mu[p,:]; with centered coords m'=mu-0.5
it separates into an 81-monomial feature contraction Q = G[i,:].F[p,:] on
the TensorEngine.  fp32 matmul measures ~2.05us/MM on HW, so instead each
operand is split 3-way in bf16 (8+8+8 mantissa bits) and the 6 cross-term
products with combined split-level <= 2 are kept — 486 K-rows, zero-padded
to 4 K=128 parts, accumulated into each PSUM chunk.  That's ~fp32 accuracy
at bf16 speed.  MMs are issued weight-major over half-blocks (one lhsT
part swept across 4 chunks before switching) — LDWEIGHTS reuse makes MMs
~2x faster than chunk-major order (116 vs 243 ns/MM measured).

The drain is a fused Reciprocal+row-sum pass (ACT Reciprocal via raw emit,
~1.2e-5 rel err; DVE takes a share via reciprocal_approx_fast+accum).

Numerics: rows are host-sorted by s2prod ascending; each core's first
128-row block (tiny s2prod = ill-conditioned) instead uses the 2+2 split
Q = (G01.F01)*(G23.F23) (9 features, 54 packed rows), and S is floored
at 1.  Validated vs fp64: metric ~1.4e-4 (gate 2e-2).

Sharding: core k handles batch k//2, c-half k%2 (2048 rows x 4096 points).
Host un-permutes output columns.
"""

import numpy as np

B, C, D, CH = 4, 4096, 4, 256
NCORES = 8
CW = C // 2            # rows per core
NBLK = CW // 128       # 16 row blocks
NP = 4                 # packed K-parts (486 rows -> 4 x 128)
HB = NBLK // 2         # blocks per epilogue half
HC = HB * 128          # columns per epilogue half

_cache = {}


def _build(bench_nrep=None, bench_span="full", parts=None, nq=4, loads_x=1, epi_x=1):
    import concourse.bacc as bacc
    import concourse.mybir as mybir
    from concourse.tile import TileContext

    if parts is None:
        parts = {"block0", "main", "drain", "epi"}

    f32 = mybir.dt.float32
    bf = mybir.dt.bfloat16
    Alu = mybir.AluOpType
    Act = mybir.ActivationFunctionType

    nc = bacc.Bacc(None, target_bir_lowering=False)
    fp_r = nc.declare_dram_parameter("fp", [128, NP * C], bf, isOutput=False)
    gp_r = nc.declare_dram_parameter("gp", [128, NP * CW], bf, isOutput=False)
    fp0_r = nc.declare_dram_parameter("fp0", [54, 2 * C], bf, isOutput=False)
    gp0_r = nc.declare_dram_parameter("gp0", [54, 256], bf, isOutput=False)
    ps2_r = nc.declare_dram_parameter("ps2", [128, NBLK], f32, isOutput=False)
    xcol_r = nc.declare_dram_parameter("xcol", [128, 2], f32, isOutput=False)
    y = nc.declare_dram_parameter("y", [CH, CW], f32, isOutput=True)

    def recip_accum_act(in_ap, junk_ap, accum_ap):
        """ACT Reciprocal with free-dim row-sum accumulate (raw emit: the
        bass wrapper refuses Reciprocal; its table is ~1.2e-5 max rel err,
        fine for summing positive terms)."""
        eng = nc.scalar
        imm = lambda v: mybir.ImmediateValue(dtype=mybir.dt.float32, value=v)
        eng.add_instruction(
            mybir.InstActivation(
                name=nc.get_next_instruction_name(),
                func=Act.Reciprocal,
                ins=[eng.lower_ap(in_ap), imm(0.0), imm(1.0), imm(0.0)],
                outs=[eng.lower_ap(junk_ap), eng.lower_ap(accum_ap)],
            )
        )

    with TileContext(nc) as tc:
        with (
            tc.tile_pool(name="persist", bufs=1) as pp,
            tc.tile_pool(name="work", bufs=1) as wp,
            tc.tile_pool(name="psum", bufs=1, space="PSUM") as psp,
            tc.tile_pool(name="dram", bufs=1, space="DRAM") as dp,
        ):
            st = {}

            def loads():
                st["scr"] = dp.tile([CW], f32, name="scr", tag="scr", bufs=2)
                fp = st["fp"] = pp.tile([128, NP * C], bf, tag="fp", bufs=2, name="fp")
                gp = st["gp"] = pp.tile([128, NP * CW], bf, tag="gp", bufs=2, name="gp")
                fp0 = st["fp0"] = pp.tile([54, 2 * C], bf, tag="fp0", bufs=2, name="fp0")
                gp0 = st["gp0"] = pp.tile([54, 256], bf, tag="gp0", bufs=2, name="gp0")
                ps2_sb = st["ps2"] = pp.tile([128, NBLK], f32, tag="ps2s", bufs=2, name="ps2s")
                xcol = st["xcol"] = pp.tile([128, 2], f32, tag="xc", bufs=2, name="xc")
                Racc = st["Racc"] = pp.tile([128, NBLK, 4], f32, tag="Ra", bufs=2, name="Ra")
                st["Racc0"] = pp.tile([128, C // 512], f32, tag="Ra0", bufs=2, name="Ra0")
                for q in range(NP):
                    nc.sync.dma_start(
                        out=fp[:, q * C : (q + 1) * C],
                        in_=fp_r[:, q * C : (q + 1) * C],
                    )
                for q in range(2):
                    w = NP * CW // 2
                    nc.sync.dma_start(
                        out=gp[:, q * w : (q + 1) * w],
                        in_=gp_r[:, q * w : (q + 1) * w],
                    )
                nc.sync.dma_start(out=fp0[:, :], in_=fp0_r[:, :])
                nc.sync.dma_start(out=gp0[:, :], in_=gp0_r[:, :])
                nc.sync.dma_start(out=ps2_sb[:, :], in_=ps2_r[:, :])
                nc.sync.dma_start(out=xcol[:, :], in_=xcol_r[:, :])
                nc.vector.memset(Racc[:, 0, :], 0.0)

            def drain_main(qt, n, g):
                # qt: (128, 1024) PSUM group = points [g*1024, (g+1)*1024)
                # ACT-only: measured 320 ns/group; DVE recip+accum is ~2.2us
                Racc = st["Racc"]
                junkC = wp.tile([128, 1024], bf, tag="jC", bufs=2, name="jC")
                recip_accum_act(qt[:, :], junkC[:, :], Racc[:, n, g : g + 1])

            def block0():
                fp0, gp0, Racc0 = st["fp0"], st["gp0"], st["Racc0"]
                # rows 0-127 (smallest s2prod): Q = (G01.F01)*(G23.F23)
                for j in range(C // 512):
                    sl = slice(j * 512, (j + 1) * 512)
                    sl2 = slice(C + j * 512, C + (j + 1) * 512)
                    qt = psp.tile([128, 1024], f32, tag="q", bufs=4, name="qp")
                    nc.tensor.matmul(
                        qt[:, 0:512], gp0[:, 0:128], fp0[:, sl],
                        start=True, stop=True,
                    )
                    nc.tensor.matmul(
                        qt[:, 512:1024], gp0[:, 128:256], fp0[:, sl2],
                        start=True, stop=True,
                    )
                    if "drain" in parts:
                        rA = wp.tile([128, 512], f32, tag="rA", bufs=2, name="rA")
                        junkA = wp.tile([128, 512], f32, tag="jA", bufs=2, name="jA")
                        recip_accum_act(qt[:, 0:512], rA[:, :], junkA[:, 0:1])
                        rB = wp.tile([128, 512], f32, tag="rB", bufs=2, name="rB")
                        nc.vector.reciprocal_approx_fast(
                            out=rB[:, :], in_=qt[:, 512:1024]
                        )
                        junkB = wp.tile([128, 512], f32, tag="jB", bufs=2, name="jB")
                        nc.vector.affine_mul_reduce(
                            out=junkB[:, :], accum_out=Racc0[:, j : j + 1],
                            in0=rA[:, :], in1=rB[:, :], scale=1.0, bias=0.0,
                        )

            def main_blocks(n_lo, n_hi):
                fp, gp = st["fp"], st["gp"]
                for n in range(n_lo, n_hi):
                    for hb in range(2):
                        # half-block: 4 chunks of 512 points = 2 PSUM tiles
                        pss = [
                            psp.tile([128, 1024], f32, tag="q", bufs=4, name="qt")
                            for _ in range(2)
                        ]
                        for q in range(nq):
                            gsl = slice(q * CW + n * 128, q * CW + (n + 1) * 128)
                            for j in range(4):
                                j0 = hb * 4 + j
                                nc.tensor.matmul(
                                    pss[j // 2][:, (j % 2) * 512 : (j % 2 + 1) * 512],
                                    gp[:, gsl],
                                    fp[:, q * C + j0 * 512 : q * C + (j0 + 1) * 512],
                                    start=(q == 0),
                                    stop=(q == nq - 1),
                                )
                        if "drain" in parts:
                            drain_main(pss[0], n, hb * 2)
                            drain_main(pss[1], n, hb * 2 + 1)

            def epilogue(half):
                scr, Racc, Racc0 = st["scr"], st["Racc"], st["Racc0"]
                ps2_sb, xcol = st["ps2"], st["xcol"]
                nsl = slice(half * HB, (half + 1) * HB)
                t1 = wp.tile([128, HB], f32, tag="t1", bufs=2, name="t1")
                nc.vector.tensor_tensor(
                    t1[:, :], Racc[:, nsl, 0], Racc[:, nsl, 1], Alu.add
                )
                t2 = wp.tile([128, HB], f32, tag="t2", bufs=2, name="t2")
                nc.vector.tensor_tensor(
                    t2[:, :], Racc[:, nsl, 2], Racc[:, nsl, 3], Alu.add
                )
                Rsum = wp.tile([128, HB], f32, tag="Rs", bufs=2, name="Rs")
                nc.vector.tensor_tensor(Rsum[:, :], t1[:, :], t2[:, :], Alu.add)
                if half == 0:
                    jr = wp.tile([128, C // 512], f32, tag="jr", bufs=2, name="jr")
                    nc.vector.tensor_scalar(
                        jr[:, :], Racc0[:, :], 0.0, None, Alu.add, Alu.add,
                        accum_out=Rsum[:, 0:1],
                    )
                S = wp.tile([128, HB], f32, tag="S", bufs=2, name="S")
                nc.vector.tensor_tensor(S[:, :], Rsum[:, :], ps2_sb[:, nsl], Alu.mult)
                nc.vector.tensor_scalar_max(S[:, :], S[:, :], 1.0)
                coef = wp.tile([128, HB], f32, tag="coef", bufs=2, name="coef")
                nc.vector.reciprocal(coef[:, :], S[:, :])

                # transpose (128, HB) -> c-ordered row via DRAM bounce
                nc.sync.dma_start(
                    out=scr.rearrange("(n p) -> p n", p=128)[:, nsl], in_=coef[:, :]
                )
                cbc = wp.tile([128, HC], f32, tag="cbc", bufs=2, name="cbc")
                nc.sync.dma_start(
                    out=cbc[:, :],
                    in_=scr.rearrange("(one c) -> one c", one=1)[
                        0:1, half * HC : (half + 1) * HC
                    ].broadcast_to([128, HC]),
                )
                for h in range(CH // 128):
                    zt = wp.tile([128, HC], f32, tag="zt", bufs=2, name="zt")
                    nc.vector.tensor_scalar_mul(zt[:, :], cbc[:, :], xcol[:, h : h + 1])
                    nc.sync.dma_start(
                        out=y[h * 128 : (h + 1) * 128, half * HC : (half + 1) * HC],
                        in_=zt[:, :],
                    )

            def whole():
                for _lx in range(loads_x):
                    loads()
                if "block0" in parts:
                    block0()
                if "main" in parts:
                    main_blocks(1, HB)
                if "epi" in parts and "drain" in parts:
                    for _ex in range(epi_x):
                        epilogue(0)
                if "main" in parts:
                    main_blocks(HB, NBLK)
                if "epi" in parts and "drain" in parts:
                    for _ex in range(epi_x):
                        epilogue(1)

            if bench_nrep is None:
                whole()
            else:
                import concourse.mybir as _mb

                with tc.For_i(
                    0, bench_nrep, 1,
                    staggered_reset=True,
                    hint_engines=(_mb.EngineType.DVE, _mb.EngineType.Activation),
                ):
                    whole()
    nc.finalize()
    return nc


def _get_nc():
    if "nc" not in _cache:
        _cache["nc"] = _build()
    return _cache["nc"]


_IDX4 = np.indices((3, 3, 3, 3)).reshape(4, -1).T  # (81, 4) exponent tuples
_IDX2 = np.indices((3, 3)).reshape(2, -1).T        # (9, 2)
_COMBOS = [(0, 0), (0, 1), (1, 0), (0, 2), (1, 1), (2, 0)]  # split levels i+j<=2


def _feat(m, s2, dims, idx):
    """G (rows, nf), F (points, nf) in float64 for the given dims."""
    n = m.shape[0]
    G = np.ones((n, len(idx)))
    F = np.ones((n, len(idx)))
    for e, exps in enumerate(idx):
        for d, ed in zip(dims, exps):
            gd = [s2[:, d] + m[:, d] ** 2, -2.0 * m[:, d], np.ones(n)][ed]
            fd = [np.ones(n), m[:, d], m[:, d] ** 2][ed]
            G[:, e] = G[:, e] * gd
            F[:, e] = F[:, e] * fd
    return G, F


def _bf16(a):
    bits = np.asarray(a, np.float32).view(np.uint32)
    r = ((bits.astype(np.uint64) + 0x7FFF + ((bits >> 16) & 1)) >> 16) << 16
    return r.astype(np.uint32).view(np.float32)


def _split3(a):
    a = np.asarray(a, np.float32)
    h = _bf16(a)
    m = _bf16((a - h).astype(np.float32))
    l = _bf16((a - h - m).astype(np.float32))
    return h, m, l


def _pack6(G, F):
    """bf16 3-way split, 6 cross-terms: (rows, 6nf), (points, 6nf)."""
    Gs = _split3(G.astype(np.float32))
    Fs = _split3(F.astype(np.float32))
    Gp = np.concatenate([Gs[i] for i, j in _COMBOS], axis=1)
    Fp = np.concatenate([Fs[j] for i, j in _COMBOS], axis=1)
    return Gp, Fp


def _to_parts(a, nrows, width):
    """(K, width) -> zero-pad K to NP*128 -> (128, NP*width) bf16."""
    import ml_dtypes

    pad = np.zeros((NP * 128, width), np.float32)
    pad[:nrows] = a
    return np.ascontiguousarray(
        pad.reshape(NP, 128, width).transpose(1, 0, 2).reshape(128, NP * width)
    ).astype(ml_dtypes.bfloat16)


def _in_maps(x, mu, sig):
    import ml_dtypes

    maps = []
    perms = []
    for k in range(NCORES):
        b, half = k // 2, k % 2
        m = mu[b].astype(np.float64) - 0.5        # centered, (C, D)
        s2 = sig[b].astype(np.float64) ** 2
        s2p = s2.prod(axis=1)
        order = np.argsort(
            s2p[half * CW : (half + 1) * CW], kind="stable"
        )  # within-half ascending s2prod
        rows = half * CW + order
        G, F = _feat(m, s2, (0, 1, 2, 3), _IDX4)
        G01, F01 = _feat(m, s2, (0, 1), _IDX2)
        G23, F23 = _feat(m, s2, (2, 3), _IDX2)
        r0 = rows[:128]
        Gp, Fp = _pack6(G[rows], F)               # (2048, 486), (4096, 486)
        Gp0, Fp0 = _pack6(G01[r0], F01)           # (128, 54), (4096, 54)
        Gp2, Fp2 = _pack6(G23[r0], F23)
        maps.append(
            {
                "fp": _to_parts(Fp.T, 486, C),
                "gp": _to_parts(Gp.T, 486, CW),
                "fp0": np.ascontiguousarray(
                    np.concatenate([Fp0.T, Fp2.T], axis=1)
                ).astype(ml_dtypes.bfloat16),
                "gp0": np.ascontiguousarray(
                    np.concatenate([Gp0.T, Gp2.T], axis=1)
                ).astype(ml_dtypes.bfloat16),
                "ps2": np.ascontiguousarray(
                    s2p[rows].reshape(NBLK, 128).T, np.float32
                ),
                "xcol": np.ascontiguousarray(
                    x[b, :, 0].reshape(2, 128).T, np.float32
                ),
            }
        )
        perms.append(order)
    return maps, perms


def kernel(x, pi, mu, sig):
    from concourse.bass_utils import run_bass_kernel_spmd

    nc = _get_nc()
    maps, perms = _in_maps(x, mu, sig)
    res = run_bass_kernel_spmd(nc, maps, list(range(NCORES))).results
    y = np.empty((B, CH, C), np.float32)
    for k in range(NCORES):
        b, half = k // 2, k % 2
        y[b][:, half * CW + perms[k]] = res[k]["y"]
    return y
